# revision 8
# baseline (speedup 1.0000x reference)
"""Trainium2 Bass kernel for nn_ConvNet pooling problem.

Pipeline per batch image (on each of 8 cores, 4 batches/core, data parallel):
  h    = relu(W1' @ x + bias')          # BN folded into W1/bias on host
  mask = sigmoid(W2 @ h + b2)           # (P, HW), accum_out -> sum_w partials
  vec  = (x @ mask^T) / sum_w           # contract over HW via PE transposes

Self-contained: hardcodes shapes/sharding; only imports the trn toolchain.
"""

import sys

sys.path.insert(0, "/opt/trn_rl_repo")

from contextlib import ExitStack

import numpy as np

import concourse.bass as bass
import concourse.bacc as bacc
import concourse.mybir as mybir
import concourse.tile as tile
from concourse.bass_utils import run_bass_kernel_spmd

B, C, P, H, W = 32, 256, 8, 64, 64
HW = H * W
NCORES = 8
BPC = B // NCORES  # batches per core
BN_EPS = 1e-5

F32 = mybir.dt.float32
F32R = mybir.dt.float32r
AF = mybir.ActivationFunctionType

KC = 128          # contraction block (partition dim)
NB = C // KC      # 2 channel blocks
CH = 512          # hw chunk for conv1/conv2 (fp32 moving-operand max)
NCH = HW // CH    # 8
TCH = 128         # hw chunk for transposes / vec matmul
NTCH = HW // TCH  # 32


def _emit(ctx: ExitStack, tc: tile.TileContext, nc: bass.Bass, d):
    wpool = ctx.enter_context(tc.tile_pool(name="weights", bufs=1))
    xpool = ctx.enter_context(tc.tile_pool(name="x", bufs=4))
    hpool = ctx.enter_context(tc.tile_pool(name="h", bufs=4))
    mpool = ctx.enter_context(tc.tile_pool(name="mask", bufs=2))
    mtpool = ctx.enter_context(tc.tile_pool(name="maskT", bufs=2))
    xtpool = ctx.enter_context(tc.tile_pool(name="xT", bufs=3))
    vpool = ctx.enter_context(tc.tile_pool(name="vec", bufs=2))
    spool = ctx.enter_context(tc.tile_pool(name="sums", bufs=2))

    ps1 = ctx.enter_context(tc.tile_pool(name="ps1", bufs=2, space="PSUM"))
    ps2 = ctx.enter_context(tc.tile_pool(name="ps2", bufs=2, space="PSUM"))
    psmt = ctx.enter_context(tc.tile_pool(name="psmt", bufs=1, space="PSUM"))
    psxt = ctx.enter_context(tc.tile_pool(name="psxt", bufs=2, space="PSUM"))
    psv = ctx.enter_context(tc.tile_pool(name="psv", bufs=1, space="PSUM"))

    # ---- load constants / weights ----
    w1t_blocks = []
    for cb in range(NB):
        t = wpool.tile([KC, C], F32R, tag=f"w1t{cb}", name=f"w1t_{cb}")
        nc.sync.dma_start(t[:], d["w1t"].ap()[cb * KC:(cb + 1) * KC, :])
        w1t_blocks.append(t)
    w2t_blocks = []
    for cb in range(NB):
        t = wpool.tile([KC, P], F32R, tag=f"w2t{cb}", name=f"w2t_{cb}")
        nc.sync.dma_start(t[:], d["w2t"].ap()[cb * KC:(cb + 1) * KC, :])
        w2t_blocks.append(t)
    b1_blocks = []
    for ob in range(NB):
        t = wpool.tile([KC, 1], F32, tag=f"b1{ob}", name=f"b1_{ob}")
        nc.sync.dma_start(t[:], d["b1"].ap()[ob * KC:(ob + 1) * KC, :])
        b1_blocks.append(t)
    b2_t = wpool.tile([P, 1], F32, tag="b2")
    nc.sync.dma_start(b2_t[:], d["b2"].ap()[:, :])
    id128 = wpool.tile([128, 128], F32R, tag="id128")
    nc.sync.dma_start(id128[:], d["id128"].ap()[:, :])
    id8 = wpool.tile([P, P], F32R, tag="id8")
    nc.sync.dma_start(id8[:], d["id8"].ap()[:, :])

    for b in range(BPC):
        # ---- load x (2 channel blocks of (128, 4096)) ----
        x_blocks = []
        for cb in range(NB):
            t = xpool.tile([KC, HW], F32R, tag="x", name=f"x_{b}_{cb}")
            nc.sync.dma_start(t[:], d["x"].ap()[b, cb * KC:(cb + 1) * KC, :])
            x_blocks.append(t)

        # ---- conv1 + BN + relu -> h (2 blocks of (128, 4096)) ----
        h_blocks = [hpool.tile([KC, HW], F32R, tag="h", name=f"h_{b}_{i}") for i in range(NB)]
        for ob in range(NB):
            for k in range(NCH):
                ps = ps1.tile([KC, CH], F32, tag="ps1", name=f"ps1_{b}_{ob}_{k}")
                for cb in range(NB):
                    nc.tensor.matmul(
                        ps[:],
                        lhsT=w1t_blocks[cb][:, ob * KC:(ob + 1) * KC],
                        rhs=x_blocks[cb][:, k * CH:(k + 1) * CH],
                        start=(cb == 0),
                        stop=(cb == NB - 1),
                    )
                nc.scalar.activation(
                    h_blocks[ob][:, k * CH:(k + 1) * CH], ps[:],
                    AF.Relu, bias=b1_blocks[ob][:, 0:1],
                )

        # ---- conv2 + sigmoid -> mask (8, 4096); accum partial sums ----
        mask_t = mpool.tile([P, HW], F32R, tag="mask", name=f"mask_{b}")
        msums = spool.tile([P, NCH], F32, tag="msums", name=f"msums_{b}")
        for k in range(NCH):
            ps = ps2.tile([P, CH], F32, tag="ps2", name=f"ps2_{b}_{k}")
            for cb in range(NB):
                nc.tensor.matmul(
                    ps[:],
                    lhsT=w2t_blocks[cb][:],
                    rhs=h_blocks[cb][:, k * CH:(k + 1) * CH],
                    start=(cb == 0),
                    stop=(cb == NB - 1),
                )
            nc.scalar.activation(
                mask_t[:, k * CH:(k + 1) * CH], ps[:],
                AF.Sigmoid, bias=b2_t[:, 0:1],
                accum_out=msums[:, k:k + 1],
            )

        # ---- sum_w and reciprocal ----
        rcp = spool.tile([P, 1], F32, tag="rcp", name=f"rcp_{b}")
        sumw = spool.tile([P, 1], F32, tag="sumw", name=f"sumw_{b}")
        nc.vector.tensor_reduce(
            sumw[:], msums[:], axis=mybir.AxisListType.X, op=mybir.AluOpType.add
        )
        nc.vector.reciprocal(rcp[:], sumw[:])

        # ---- maskT: PE-transpose all 32 chunks into one PSUM bank ----
        mt_ps = psmt.tile([128, NTCH * P], F32R, tag="mtps", name=f"mtps_{b}")
        for k in range(NTCH):
            nc.tensor.transpose(
                mt_ps[:, k * P:(k + 1) * P],
                mask_t[:, k * TCH:(k + 1) * TCH],
                id8[:],
            )
        maskT_sb = mtpool.tile([128, NTCH * P], F32R, tag="maskT", name=f"maskT_{b}")
        nc.vector.tensor_copy(maskT_sb[:], mt_ps[:])

        # ---- xT via PE transpose + vec matmul (accumulate over hw chunks) ----
        vec_ps = psv.tile([P, C], F32, tag="vps", name=f"vps_{b}")
        for kk in range(NTCH // 2):
            psx = psxt.tile([128, 2 * C], F32R, tag="psx", name=f"psx_{b}_{kk}")
            for j in range(2):
                k = 2 * kk + j
                for cb in range(NB):
                    nc.tensor.transpose(
                        psx[:, j * C + cb * KC: j * C + (cb + 1) * KC],
                        x_blocks[cb][:, k * TCH:(k + 1) * TCH],
                        id128[:],
                    )
            xt_sb = xtpool.tile([128, 2 * C], F32R, tag="xt", name=f"xt_{b}_{kk}")
            nc.vector.tensor_copy(xt_sb[:], psx[:])
            for j in range(2):
                k = 2 * kk + j
                nc.tensor.matmul(
                    vec_ps[:],
                    lhsT=maskT_sb[:, k * P:(k + 1) * P],
                    rhs=xt_sb[:, j * C:(j + 1) * C],
                    start=(k == 0),
                    stop=(k == NTCH - 1),
                )

        # ---- normalize and store: out[b] = vecT (P, C) ----
        vec_sb = vpool.tile([P, C], F32, tag="vec", name=f"vec_{b}")
        nc.vector.tensor_scalar_mul(vec_sb[:], vec_ps[:], rcp[:, 0:1])
        nc.sync.dma_start(d["out"].ap()[b], vec_sb[:])


def build_nc() -> bass.Bass:
    nc = bacc.Bacc("TRN2", target_bir_lowering=False, debug=False)
    d = {
        "x": nc.dram_tensor("x", [BPC, C, HW], F32R, kind="ExternalInput"),
        "w1t": nc.dram_tensor("w1t", [C, C], F32R, kind="ExternalInput"),
        "b1": nc.dram_tensor("b1", [C, 1], F32, kind="ExternalInput"),
        "w2t": nc.dram_tensor("w2t", [C, P], F32R, kind="ExternalInput"),
        "b2": nc.dram_tensor("b2", [P, 1], F32, kind="ExternalInput"),
        "id128": nc.dram_tensor("id128", [128, 128], F32R, kind="ExternalInput"),
        "id8": nc.dram_tensor("id8", [P, P], F32R, kind="ExternalInput"),
        "out": nc.dram_tensor("out", [BPC, P, C], F32, kind="ExternalOutput"),
    }
    with tile.TileContext(nc) as tc, ExitStack() as ctx:
        _emit(ctx, tc, nc, d)
    nc.compile()
    return nc


_NC_CACHE = None


def _get_nc():
    global _NC_CACHE
    if _NC_CACHE is None:
        _NC_CACHE = build_nc()
    return _NC_CACHE


def _prep_in_maps(x, W1, b1, gamma, beta, mean, var, W2, b2):
    x = np.asarray(x, dtype=np.float32)
    W1 = np.asarray(W1, dtype=np.float32)
    b1 = np.asarray(b1, dtype=np.float32)
    gamma = np.asarray(gamma, dtype=np.float32)
    beta = np.asarray(beta, dtype=np.float32)
    mean = np.asarray(mean, dtype=np.float32)
    var = np.asarray(var, dtype=np.float32)
    W2 = np.asarray(W2, dtype=np.float32)
    b2 = np.asarray(b2, dtype=np.float32)

    inv = gamma / np.sqrt(var + BN_EPS)
    W1f = W1 * inv[:, None]                      # (o, c): fold BN scale
    biasf = b1 * inv + beta - mean * inv         # (o,)
    w1t = np.ascontiguousarray(W1f.T)            # (c_in, c_out) lhsT layout
    w2t = np.ascontiguousarray(W2.T)             # (c, p) lhsT layout
    id128 = np.eye(128, dtype=np.float32)
    id8 = np.eye(P, dtype=np.float32)

    xs = x.reshape(NCORES, BPC, C, HW)
    shared = {
        "w1t": w1t,
        "b1": np.ascontiguousarray(biasf.reshape(C, 1)),
        "w2t": w2t,
        "b2": np.ascontiguousarray(b2.reshape(P, 1)),
        "id128": id128,
        "id8": id8,
    }
    return [
        {"x": np.ascontiguousarray(xs[i]), **shared} for i in range(NCORES)
    ]


def run(inputs: dict, trace: bool = False):
    """Run the bass kernel; returns (full_output, BassKernelResults)."""
    in_maps = _prep_in_maps(**inputs)
    nc = _get_nc()
    try:
        res = run_bass_kernel_spmd(
            nc, in_maps, core_ids=list(range(NCORES)), trace=trace
        )
    except ModuleNotFoundError:
        if not trace:
            raise
        res = run_bass_kernel_spmd(
            nc, in_maps, core_ids=list(range(NCORES)), trace=False
        )
    outs = np.stack([r["out"] for r in res.results])   # (8, 4, P, C) = vecT
    vecT = outs.reshape(B, P, C)
    vec = vecT.transpose(0, 2, 1)                      # (B, C, P)
    full = np.ascontiguousarray(vec.reshape(B, P, C)).astype(np.float32)
    return full, res


def kernel(**inputs) -> np.ndarray:
    out, _ = run(inputs, trace=False)
    return out


# revision 19
# speedup vs baseline: 19192.6834x; 19192.6834x over previous
"""Trainium2 Bass kernel for nn_ConvNet pooling problem.

Pipeline per batch image (on each of 8 cores, 4 batches/core, data parallel):
  h    = relu(W1' @ x + bias')          # BN folded into W1/bias on host
  mask = sigmoid(W2 @ h + b2)           # (P, HW), accum_out -> sum_w partials
  vec  = (x @ mask^T) / sum_w           # contract over HW via PE transposes

Self-contained: hardcodes shapes/sharding; only imports the trn toolchain.
"""

import sys

sys.path.insert(0, "/opt/trn_rl_repo")

from contextlib import ExitStack

import numpy as np

import concourse.bass as bass
import concourse.bacc as bacc
import concourse.mybir as mybir
import concourse.tile as tile
from concourse.bass_utils import run_bass_kernel_spmd

B, C, P, H, W = 32, 256, 8, 64, 64
HW = H * W
NCORES = 8
BPC = B // NCORES  # batches per core
BN_EPS = 1e-5

F32 = mybir.dt.float32
F32R = mybir.dt.float32r
AF = mybir.ActivationFunctionType

KC = 128          # contraction block (partition dim)
NB = C // KC      # 2 channel blocks
CH = 512          # hw chunk for conv1/conv2 (fp32 moving-operand max)
NCH = HW // CH    # 8
TCH = 128         # hw chunk for transposes / vec matmul
NTCH = HW // TCH  # 32


def _emit(ctx: ExitStack, tc: tile.TileContext, nc: bass.Bass, d):
    wpool = ctx.enter_context(tc.tile_pool(name="weights", bufs=1))
    xpool = ctx.enter_context(tc.tile_pool(name="x", bufs=4))
    hpool = ctx.enter_context(tc.tile_pool(name="h", bufs=3))
    mpool = ctx.enter_context(tc.tile_pool(name="mask", bufs=2))
    mtpool = ctx.enter_context(tc.tile_pool(name="maskT", bufs=2))
    xtpool = ctx.enter_context(tc.tile_pool(name="xT", bufs=18))
    vpool = ctx.enter_context(tc.tile_pool(name="vec", bufs=2))
    spool = ctx.enter_context(tc.tile_pool(name="sums", bufs=2))

    ps1 = ctx.enter_context(tc.tile_pool(name="ps1", bufs=2, space="PSUM"))
    ps2 = ctx.enter_context(tc.tile_pool(name="ps2", bufs=2, space="PSUM"))
    psmt = ctx.enter_context(tc.tile_pool(name="psmt", bufs=1, space="PSUM"))
    psxt = ctx.enter_context(tc.tile_pool(name="psxt", bufs=2, space="PSUM"))
    psv = ctx.enter_context(tc.tile_pool(name="psv", bufs=1, space="PSUM"))

    # ---- batch-0 x load first (pipeline fill), then weights ----
    x_pre = []
    for cb in range(NB):
        t = xpool.tile([KC, HW], F32R, tag="x", name=f"x_0_{cb}")
        for q in range(4):
            nc.sync.dma_start(
                t[:, q * (HW // 4):(q + 1) * (HW // 4)],
                d["x"].ap()[0, cb * KC:(cb + 1) * KC,
                            q * (HW // 4):(q + 1) * (HW // 4)],
            )
        x_pre.append(t)

    # ---- load constants / weights ----
    w1t_blocks = []
    for cb in range(NB):
        t = wpool.tile([KC, C], F32R, tag=f"w1t{cb}", name=f"w1t_{cb}")
        nc.sync.dma_start(t[:], d["w1t"].ap()[cb * KC:(cb + 1) * KC, :])
        w1t_blocks.append(t)
    w2t_blocks = []
    for cb in range(NB):
        t = wpool.tile([KC, P], F32R, tag=f"w2t{cb}", name=f"w2t_{cb}")
        nc.sync.dma_start(t[:], d["w2t"].ap()[cb * KC:(cb + 1) * KC, :])
        w2t_blocks.append(t)
    b1_blocks = []
    for ob in range(NB):
        t = wpool.tile([KC, 1], F32, tag=f"b1{ob}", name=f"b1_{ob}")
        nc.sync.dma_start(t[:], d["b1"].ap()[ob * KC:(ob + 1) * KC, :])
        b1_blocks.append(t)
    b2_t = wpool.tile([P, 1], F32, tag="b2")
    nc.sync.dma_start(b2_t[:], d["b2"].ap()[:, :])
    id128 = wpool.tile([128, 128], F32R, tag="id128")
    nc.sync.dma_start(id128[:], d["id128"].ap()[:, :])
    id8 = wpool.tile([P, P], F32R, tag="id8")
    nc.sync.dma_start(id8[:], d["id8"].ap()[:, :])

    for b in range(BPC):
        # ---- load x (2 channel blocks of (128, 4096), quarter DMAs) ----
        if b == 0:
            x_blocks = x_pre
        else:
            x_blocks = []
            for cb in range(NB):
                t = xpool.tile([KC, HW], F32R, tag="x", name=f"x_{b}_{cb}")
                for q in range(4):
                    nc.sync.dma_start(
                        t[:, q * (HW // 4):(q + 1) * (HW // 4)],
                        d["x"].ap()[b, cb * KC:(cb + 1) * KC,
                                    q * (HW // 4):(q + 1) * (HW // 4)],
                    )
                x_blocks.append(t)

        # ---- conv1 (+ xT production interleaved) ----
        h_blocks = [hpool.tile([KC, HW], F32R, tag="h", name=f"h_{b}_{i}") for i in range(NB)]
        xt_tiles = []
        for k in range(NCH):
            for ob in range(NB):
                ps = ps1.tile([KC, CH], F32, tag="ps1", name=f"ps1_{b}_{ob}_{k}")
                for cb in range(NB):
                    nc.tensor.matmul(
                        ps[:],
                        lhsT=w1t_blocks[cb][:, ob * KC:(ob + 1) * KC],
                        rhs=x_blocks[cb][:, k * CH:(k + 1) * CH],
                        start=(cb == 0),
                        stop=(cb == NB - 1),
                    )
                if (k + ob) % 2 == 0:
                    nc.scalar.activation(
                        h_blocks[ob][:, k * CH:(k + 1) * CH], ps[:],
                        AF.Relu, bias=b1_blocks[ob][:, 0:1],
                    )
                else:
                    # DVE: relu(x + bias) = max(x + bias, 0) in one op
                    nc.vector.tensor_scalar(
                        h_blocks[ob][:, k * CH:(k + 1) * CH], ps[:],
                        scalar1=b1_blocks[ob][:, 0:1], scalar2=0.0,
                        op0=mybir.AluOpType.add, op1=mybir.AluOpType.max,
                    )
            # produce 2 xT double-chunks (4 hw-chunks of 128) per conv1 chunk
            for kk in (2 * k, 2 * k + 1):
                psx = psxt.tile([128, 2 * C], F32R, tag="psx", name=f"psx_{b}_{kk}")
                for j in range(2):
                    kt = 2 * kk + j
                    for cb in range(NB):
                        nc.tensor.transpose(
                            psx[:, j * C + cb * KC: j * C + (cb + 1) * KC],
                            x_blocks[cb][:, kt * TCH:(kt + 1) * TCH],
                            id128[:],
                        )
                xt_sb = xtpool.tile([128, 2 * C], F32R, tag="xt", name=f"xt_{b}_{kk}")
                if kk % 3 == 1:
                    nc.scalar.copy(xt_sb[:], psx[:])
                else:
                    nc.vector.tensor_copy(xt_sb[:], psx[:])
                xt_tiles.append(xt_sb)

        # ---- conv2 + sigmoid -> mask (8, 4096); accum partial sums ----
        mask_t = mpool.tile([P, HW], F32R, tag="mask", name=f"mask_{b}")
        msums = spool.tile([P, NCH], F32, tag="msums", name=f"msums_{b}")
        for k in range(NCH):
            ps = ps2.tile([P, CH], F32, tag="ps2", name=f"ps2_{b}_{k}")
            for cb in range(NB):
                nc.tensor.matmul(
                    ps[:],
                    lhsT=w2t_blocks[cb][:],
                    rhs=h_blocks[cb][:, k * CH:(k + 1) * CH],
                    start=(cb == 0),
                    stop=(cb == NB - 1),
                )
            nc.scalar.activation(
                mask_t[:, k * CH:(k + 1) * CH], ps[:],
                AF.Sigmoid, bias=b2_t[:, 0:1],
                accum_out=msums[:, k:k + 1],
            )

        # ---- sum_w and reciprocal ----
        rcp = spool.tile([P, 1], F32, tag="rcp", name=f"rcp_{b}")
        sumw = spool.tile([P, 1], F32, tag="sumw", name=f"sumw_{b}")
        nc.vector.tensor_reduce(
            sumw[:], msums[:], axis=mybir.AxisListType.X, op=mybir.AluOpType.add
        )
        nc.vector.reciprocal(rcp[:], sumw[:])

        # ---- maskT: PE-transpose all 32 chunks into one PSUM bank ----
        mt_ps = psmt.tile([128, NTCH * P], F32R, tag="mtps", name=f"mtps_{b}")
        for k in range(NTCH):
            nc.tensor.transpose(
                mt_ps[:, k * P:(k + 1) * P],
                mask_t[:, k * TCH:(k + 1) * TCH],
                id8[:],
            )
        maskT_sb = mtpool.tile([128, NTCH * P], F32R, tag="maskT", name=f"maskT_{b}")
        for q in range(4):
            nc.vector.tensor_copy(
                maskT_sb[:, q * 64:(q + 1) * 64], mt_ps[:, q * 64:(q + 1) * 64]
            )

        # ---- vec matmuls: accumulate over all staged xT tiles ----
        vec_ps = psv.tile([P, C], F32, tag="vps", name=f"vps_{b}")
        for kk in range(NTCH // 2):
            for j in range(2):
                k = 2 * kk + j
                nc.tensor.matmul(
                    vec_ps[:],
                    lhsT=maskT_sb[:, k * P:(k + 1) * P],
                    rhs=xt_tiles[kk][:, j * C:(j + 1) * C],
                    start=(k == 0),
                    stop=(k == NTCH - 1),
                )

        # ---- normalize and store: out[b] = vecT (P, C) ----
        vec_sb = vpool.tile([P, C], F32, tag="vec", name=f"vec_{b}")
        nc.vector.tensor_scalar_mul(vec_sb[:], vec_ps[:], rcp[:, 0:1])
        nc.sync.dma_start(d["out"].ap()[b], vec_sb[:])


def build_nc() -> bass.Bass:
    nc = bacc.Bacc("TRN2", target_bir_lowering=False, debug=False)
    d = {
        "x": nc.dram_tensor("x", [BPC, C, HW], F32R, kind="ExternalInput"),
        "w1t": nc.dram_tensor("w1t", [C, C], F32R, kind="ExternalInput"),
        "b1": nc.dram_tensor("b1", [C, 1], F32, kind="ExternalInput"),
        "w2t": nc.dram_tensor("w2t", [C, P], F32R, kind="ExternalInput"),
        "b2": nc.dram_tensor("b2", [P, 1], F32, kind="ExternalInput"),
        "id128": nc.dram_tensor("id128", [128, 128], F32R, kind="ExternalInput"),
        "id8": nc.dram_tensor("id8", [P, P], F32R, kind="ExternalInput"),
        "out": nc.dram_tensor("out", [BPC, P, C], F32, kind="ExternalOutput"),
    }
    with tile.TileContext(nc) as tc, ExitStack() as ctx:
        _emit(ctx, tc, nc, d)
    nc.compile()
    return nc


_NC_CACHE = None


def _get_nc():
    global _NC_CACHE
    if _NC_CACHE is None:
        _NC_CACHE = build_nc()
    return _NC_CACHE


def _prep_in_maps(x, W1, b1, gamma, beta, mean, var, W2, b2):
    x = np.asarray(x, dtype=np.float32)
    W1 = np.asarray(W1, dtype=np.float32)
    b1 = np.asarray(b1, dtype=np.float32)
    gamma = np.asarray(gamma, dtype=np.float32)
    beta = np.asarray(beta, dtype=np.float32)
    mean = np.asarray(mean, dtype=np.float32)
    var = np.asarray(var, dtype=np.float32)
    W2 = np.asarray(W2, dtype=np.float32)
    b2 = np.asarray(b2, dtype=np.float32)

    inv = gamma / np.sqrt(var + BN_EPS)
    W1f = W1 * inv[:, None]                      # (o, c): fold BN scale
    biasf = b1 * inv + beta - mean * inv         # (o,)
    w1t = np.ascontiguousarray(W1f.T)            # (c_in, c_out) lhsT layout
    w2t = np.ascontiguousarray(W2.T)             # (c, p) lhsT layout
    id128 = np.eye(128, dtype=np.float32)
    id8 = np.eye(P, dtype=np.float32)

    xs = x.reshape(NCORES, BPC, C, HW)
    shared = {
        "w1t": w1t,
        "b1": np.ascontiguousarray(biasf.reshape(C, 1)),
        "w2t": w2t,
        "b2": np.ascontiguousarray(b2.reshape(P, 1)),
        "id128": id128,
        "id8": id8,
    }
    return [
        {"x": np.ascontiguousarray(xs[i]), **shared} for i in range(NCORES)
    ]


def run(inputs: dict, trace: bool = False):
    """Run the bass kernel; returns (full_output, BassKernelResults)."""
    in_maps = _prep_in_maps(**inputs)
    nc = _get_nc()
    res = None
    last_exc = None
    for attempt in range(3):
        try:
            res = run_bass_kernel_spmd(
                nc, in_maps, core_ids=list(range(NCORES)), trace=trace
            )
            break
        except ModuleNotFoundError:
            # axon NTFF profiling hook unavailable in this container
            trace = False
            continue
        except Exception as e:  # transient device/runtime hiccups: retry
            last_exc = e
            import time as _t

            _t.sleep(5.0 * (attempt + 1))
            continue
    if res is None:
        raise last_exc
    outs = np.stack([r["out"] for r in res.results])   # (8, 4, P, C) = vecT
    vecT = outs.reshape(B, P, C)
    vec = vecT.transpose(0, 2, 1)                      # (B, C, P)
    full = np.ascontiguousarray(vec.reshape(B, P, C)).astype(np.float32)
    return full, res


def kernel(**inputs) -> np.ndarray:
    out, _ = run(inputs, trace=False)
    return out
